# revision 26
# baseline (speedup 1.0000x reference)
"""BackgroundForegroundNeRF fused MLP kernel for 8x Trainium2 NeuronCores.

Pure data parallel: the 2M points are split across 8 cores; all weights are
replicated. Per core the network runs feature-major ([feature, point] tiles)
with every matmul in single fp16 (1 cyc/row on the PE vs 4 for fp32; the
2e-2 harness tolerance dwarfs fp16's ~1e-3 accumulated error):

  x is transposed to feature-major fp16 on the HOST and DMA'd directly.
  MM1  : W1 (bg_s0 zero-padded | fg_s0 blocks)       -> h1  [128, n]   relu
  MM2  : block-diag(bg_s1, fg_s1)                    -> h2  [128, n]   relu
  C0   : views-part (base-64 zero-padded lhsT, straight from xT)
         + (c0_geo @ s2_geo) @ h2 (geo path folded into one matrix - no
         relu between the sigma-net output and the color-net input)  relu
  C1,C2: block-diag color layers                     relu
  HEADS: the sigma head (w3: 128->3) and color head (wc3: 128->6) are
         emitted activation-stationary per 128-point block:
            matmul(out=[128pt, 3], lhsT=h2_block[128ch,128pt], rhs=w3)
         so the tiny heads cost N=3/6 moving rows instead of a full
         2048-row pass each, AND land directly point-major in PSUM -
         no PE transposes and no [9, n] psum->sbuf copy at all.
  Blend (softplus sigma weights, sigma-weighted colors) runs point-major
  on [128, PPB, k] tiles, then DMA out [n, 6].

Softplus is composed as ln(exp(x) + 1) on the ACT engine (this container's
act tables have no native softplus). relu psum->sbuf copies rotate across
ACT/DVE(/Pool) per chunk so no single engine gates the PE.
"""
import os
import sys

_HERE = os.path.dirname(os.path.abspath(__file__))
sys.path.insert(0, '/opt/trn_rl_repo')

import numpy as np  # noqa: E402

import concourse.bass as bass  # noqa: E402
import concourse.tile as tile  # noqa: E402
from concourse import mybir  # noqa: E402
from concourse.bass_utils import run_bass_kernel_spmd  # noqa: E402

F32 = mybir.dt.float32
F16 = mybir.dt.float16
AF = mybir.ActivationFunctionType

N_CORES = 8
IN_CH, IN_VIEWS, TIME_DIM, HID, GEO = 71, 27, 8, 64, 15
NF = IN_CH + IN_VIEWS            # 98
TILE_PTS = int(os.environ.get('NERF_TILE', '4096'))
PPB = TILE_PTS // 128            # points per partition
CHUNK = int(os.environ.get('NERF_CHUNK', '512'))   # relu/psum granularity
MMN = 512                        # matmul moving free dim (one PSUM bank)
PSUM_BUFS = int(os.environ.get('NERF_PSUM_BUFS', '6'))
BIGS_BUFS = int(os.environ.get('NERF_BIGS_BUFS', '2'))
IO_BUFS = int(os.environ.get('NERF_IO_BUFS', '3'))
# relu chunk engine rotation: A=ACT, V=DVE (the only two PSUM readers;
# Pool/GPSIMD and DMA cannot touch PSUM on TRN2)
RELU_ENG = os.environ.get('NERF_RELU_ENG', 'AV')

LAST_RESULT = None               # BassKernelResults of the last run (for test.py)


def _split_multiwait_instructions(nc, limit=1):
    """The walrus build here rejects instructions with >1 sync wait; hoist
    extra waits onto fresh single-wait NOPs inserted before the instruction."""
    sync_info_cls = None
    for f in nc.m.functions:
        for bb in f.blocks:
            insts = list(bb.instructions)
            if not any(
                i.sync_info is not None and i.sync_info.on_wait
                and len(i.sync_info.on_wait) > limit
                for i in insts
            ):
                continue
            new_list = []
            for inst in insts:
                si = inst.sync_info
                if si is not None and si.on_wait and len(si.on_wait) > limit:
                    if sync_info_cls is None:
                        sync_info_cls = type(si)
                    waits = list(si.on_wait)
                    keep, extra = waits[:limit], waits[limit:]
                    si.on_wait.clear()
                    si.on_wait.extend(keep)
                    for wt in extra:
                        nop = mybir.InstNoOp(
                            name=f"I-mwsplit-{nc.next_id()}", ins=[], outs=[])
                        nop.engine = inst.engine
                        nop.sync_info = sync_info_cls(on_wait=[wt], on_update=[])
                        new_list.append(nop)
                new_list.append(inst)
            while len(bb.instructions):
                bb.instructions.pop()
            for inst in new_list:
                bb.add_instruction(inst)


def _prep_weights(inp):
    """Pack the 14 small MLP weights into fused fp16 lhsT ([K, M]) matrices."""
    g = {k: np.asarray(inp[k], np.float32) for k in inp}
    z = np.zeros

    w1 = z((IN_CH, 128), np.float32)            # K=71 -> M=128 (bg|fg h1)
    w1[:63, :64] = g['bg_s0'].T                 # bg uses xyz only (63)
    w1[:71, 64:] = g['fg_s0'].T

    w2 = z((128, 128), np.float32)              # block-diag h1 -> h2
    w2[:64, :64] = g['bg_s1'].T
    w2[64:, 64:] = g['fg_s1'].T

    w3 = z((128, 3), np.float32)                # sigma/unc logits
    w3[:64, 0] = g['bg_s2'][0]                  # bg sigma
    w3[64:, 1] = g['fg_s2'][1]                  # fg uncertainty
    w3[64:, 2] = g['fg_s2'][0]                  # fg sigma (cols 4:6 of out
    #                                             are (unc, fg_sigma): after
    #                                             softplus they copy out as
    #                                             one contiguous slice)

    # c0 views part, padded so lhsT/rhs sit at base partition 64:
    # rows 64..70 (pts tail in xT) are zero, rows 71..97 are the view dirs.
    wc0e = z((NF, 128), np.float32)
    wc0e[71:, :64] = g['bg_c0'][:, :IN_VIEWS].T
    wc0e[71:, 64:] = g['fg_c0'][:, :IN_VIEWS].T

    # c0 geo part folded through the (linear) sigma-net output: geo enters
    # c0 with no relu in between, so c0_geo @ (s2_geo @ h2) collapses.
    bgp = (g['bg_c0'][:, IN_VIEWS:].astype(np.float64)
           @ g['bg_s2'][1:, :].astype(np.float64)).astype(np.float32)
    fgp = (g['fg_c0'][:, IN_VIEWS:].astype(np.float64)
           @ g['fg_s2'][2:, :].astype(np.float64)).astype(np.float32)
    wc0h = z((128, 128), np.float32)
    wc0h[:64, :64] = bgp.T
    wc0h[64:, 64:] = fgp.T

    wc1 = z((128, 128), np.float32)
    wc1[:64, :64] = g['bg_c1'].T
    wc1[64:, 64:] = g['fg_c1'].T
    wc2 = z((128, 128), np.float32)
    wc2[:64, :64] = g['bg_c2'].T
    wc2[64:, 64:] = g['fg_c2'].T

    wc3 = z((128, 6), np.float32)
    wc3[:64, 0:3] = g['bg_c3'].T
    wc3[64:, 3:6] = g['fg_c3'].T

    return {k: v.astype(np.float16) for k, v in {
        'w1': w1, 'w2': w2, 'w3': w3, 'wc0e': wc0e, 'wc0h': wc0h,
        'wc1': wc1, 'wc2': wc2, 'wc3': wc3,
    }.items()}


_PROG_CACHE = {}

_WNAMES = ['w1', 'w2', 'w3', 'wc0e', 'wc0h', 'wc1', 'wc2', 'wc3']
_WSHAPES = {'w1': [IN_CH, 128], 'w2': [128, 128], 'w3': [128, 3],
            'wc0e': [NF, 128], 'wc0h': [128, 128],
            'wc1': [128, 128], 'wc2': [128, 128], 'wc3': [128, 6]}


def _build_program(padded_pts):
    """Build the per-core Bass program for `padded_pts` points."""
    ntiles = padded_pts // TILE_PTS
    repeat = int(os.environ.get('NERF_REPEAT', '1'))
    nc = bass.Bass("TRN2", target_bir_lowering=False, debug=False,
                   num_devices=N_CORES)

    xin = nc.dram_tensor("xin", [NF, padded_pts], F16,
                         kind="ExternalInput").ap()
    out = nc.dram_tensor("out", [ntiles * 128, PPB * 6], F32,
                         kind="ExternalOutput").ap()
    wdram = {n: nc.dram_tensor(n, _WSHAPES[n], F16, kind="ExternalInput").ap()
             for n in _WNAMES}
    # NERF_REPEAT>1 is bench-only: libneuronxla's NEFF cache keys ignore the
    # embedded BIR, so give repeat variants a distinct I/O signature.
    reptag = None
    if repeat > 1:
        reptag = nc.dram_tensor("reptag", [1, repeat], F32,
                                kind="ExternalInput").ap()

    with tile.TileContext(nc) as tc:
        with tc.tile_pool(name="consts", bufs=1) as consts, \
             tc.tile_pool(name="bigs", bufs=BIGS_BUFS) as bigs, \
             tc.tile_pool(name="io", bufs=IO_BUFS) as io, \
             tc.tile_pool(name="small", bufs=2) as small, \
             tc.tile_pool(name="ps", bufs=PSUM_BUFS, space="PSUM") as ps, \
             tc.tile_pool(name="ps9", bufs=int(os.environ.get('NERF_PS9_BUFS', '2')),
                          space="PSUM") as ps9:

            W = {}
            for n in _WNAMES:
                W[n] = consts.tile(_WSHAPES[n], F16, name=f"sb_{n}")
                nc.sync.dma_start(out=W[n], in_=wdram[n])
            if reptag is not None:
                rt = consts.tile([1, repeat], F32, name="sb_reptag")
                nc.sync.dma_start(out=rt, in_=reptag)

            relu_i = [0]

            def relu_to(dst, src_psum):
                e = RELU_ENG[relu_i[0] % len(RELU_ENG)]
                relu_i[0] += 1
                if e == 'A':
                    nc.scalar.activation(out=dst, in_=src_psum, func=AF.Relu)
                else:
                    nc.vector.tensor_scalar_max(dst, src_psum, 0.0)

            nchunk = TILE_PTS // CHUNK
            gsls = [slice(ch * CHUNK, (ch + 1) * CHUNK) for ch in range(nchunk)]
            hpb = PPB // nchunk      # head blocks per chunk

            def heads(st, w, src, base, width, ch):
                # activation-stationary head matmuls for chunk ch:
                # out[128pt, width] = src_blockT @ w, directly point-major
                for j in range(ch * hpb, (ch + 1) * hpb):
                    csl = slice(j * 128, (j + 1) * 128)
                    nc.tensor.matmul(
                        st['p9'][:, j * 9 + base:j * 9 + base + width],
                        src[:, csl], w, start=True, stop=True)

            def layer(dst, w, src, extra=None, head=None):
                # one psum tile per CHUNK pts, filled by MMN-wide matmuls
                # (one PSUM bank each); optional second accumulating matmul
                # (lhsT, rhs) in `extra`; optional per-chunk head matmuls
                # interleaved to fill PE bubbles
                pss = []
                for ch in range(nchunk):
                    p = ps.tile([128, CHUNK], F32, name="p", tag="ps")
                    for s in range(CHUNK // MMN):
                        msl = slice(s * MMN, (s + 1) * MMN)
                        gsl = slice(ch * CHUNK + s * MMN,
                                    ch * CHUNK + (s + 1) * MMN)
                        nc.tensor.matmul(p[:, msl], w, src[:, gsl],
                                         start=True, stop=extra is None)
                        if extra is not None:
                            nc.tensor.matmul(p[:, msl], extra[0],
                                             extra[1][:, gsl],
                                             start=False, stop=True)
                    if head is not None:
                        heads(*head, ch)
                    pss.append(p)
                for ch in range(nchunk):
                    relu_to(dst[:, gsls[ch]], pss[ch])

            # --- software-pipelined tile stages (depth 2): PE never waits
            # on the relu drain of its own stage; stages of tile t are
            # interleaved with stages of tile t-1 ---
            def s_load(st, t):
                rows = slice(t * TILE_PTS, (t + 1) * TILE_PTS)
                st['xT'] = io.tile([NF, TILE_PTS], F16, name="xT", tag="xT")
                nc.sync.dma_start(out=st['xT'], in_=xin[:, rows])

            def s1(st):
                st['h1'] = bigs.tile([128, TILE_PTS], F16, name="h1",
                                     tag="h1")
                layer(st['h1'], W['w1'], st['xT'][0:IN_CH, :])

            def s2(st):
                st['h2'] = bigs.tile([128, TILE_PTS], F16, name="h2",
                                     tag="h2")
                layer(st['h2'], W['w2'], st['h1'])

            def s3(st):
                # p9[p, j*9+c]: c 0..2 = (bg_sig, unc, fg_sig) logits,
                #               c 3..5 = bg rgb, c 6..8 = fg rgb
                st['p9'] = ps9.tile([128, PPB * 9], F32, name="p9", tag="p9")
                st['c0'] = bigs.tile([128, TILE_PTS], F16, name="c0",
                                     tag="c0")
                layer(st['c0'], W['wc0e'][64:NF, :], st['xT'][64:NF, :],
                      extra=(W['wc0h'], st['h2']),
                      head=(st, W['w3'], st['h2'], 0, 3))

            def s4(st):
                st['c1'] = bigs.tile([128, TILE_PTS], F16, name="c1",
                                     tag="c1")
                layer(st['c1'], W['wc1'], st['c0'])

            def s5(st):
                st['c2'] = bigs.tile([128, TILE_PTS], F16, name="c2",
                                     tag="c2")
                layer(st['c2'], W['wc2'], st['c1'])

            def s6(st, t):
                for ch in range(nchunk):
                    heads(st, W['wc3'], st['c2'], 3, 6, ch)
                p9 = st['p9']
                p9r = p9.rearrange("p (j c) -> p j c", c=9)
                p92 = p9.rearrange("p (j b c) -> p j b c", b=3, c=3)

                out_sb = io.tile([128, PPB, 6], F32, name="out_sb",
                                 tag="out_sb")
                sp3 = small.tile([128, PPB, 3], F32, name="sp3", tag="sp3")
                c6 = small.tile([128, PPB, 2, 3], F32, name="c6", tag="c6")
                inv = small.tile([128, PPB], F32, name="inv", tag="inv")
                wpr = small.tile([128, PPB, 2], F32, name="wpr", tag="wpr")
                pr2 = small.tile([128, PPB, 2, 3], F32, name="pr2",
                                 tag="pr2")

                # PSUM can only be read by ACT/DVE (the relu bottleneck
                # engines), so the head PSUM leaves via two ACT ops and the
                # rest of the blend runs on the otherwise-idle Pool engine.
                # softplus(x) = ln(exp(x) + 1); exp reads PSUM directly
                nc.scalar.activation(out=sp3, in_=p9r[:, :, 0:3],
                                     func=AF.Exp)
                nc.scalar.activation(out=sp3, in_=sp3, func=AF.Ln, bias=1.0)
                nc.scalar.copy(out=c6, in_=p92[:, :, 1:3, :])
                # sigma = sp_bg + sp_fg (Pool; the reference's +1e-9 is far
                # below output tolerance, but it must guard the division)
                nc.gpsimd.tensor_add(out_sb[:, :, 3], sp3[:, :, 0],
                                     sp3[:, :, 2])
                nc.vector.tensor_scalar_add(inv, out_sb[:, :, 3], 1e-9)
                nc.vector.reciprocal(out=inv, in_=inv)
                # blend weights (w_bg, w_fg) in one strided pass (Pool)
                nc.gpsimd.tensor_mul(
                    wpr, sp3[:, :, 0:3:2],
                    inv.unsqueeze(2).broadcast_to((128, PPB, 2)))
                # pr2[p,j,k,:] = c6[p,j,k,:] * wpr[p,j,k]
                nc.gpsimd.tensor_mul(
                    pr2, c6, wpr.unsqueeze(3).broadcast_to((128, PPB, 2, 3)))
                nc.gpsimd.tensor_add(out_sb[:, :, 0:3], pr2[:, :, 0, :],
                                     pr2[:, :, 1, :])
                # (unc, fg_sigma) drop out as one contiguous copy (Pool)
                nc.gpsimd.tensor_copy(out=out_sb[:, :, 4:6],
                                      in_=sp3[:, :, 1:3])

                o_dram = out[t * 128:(t + 1) * 128, :].rearrange(
                    "p (j c) -> p j c", c=6)
                nc.sync.dma_start(out=o_dram, in_=out_sb)

            tseq = [tt for _ in range(repeat) for tt in range(ntiles)]
            prev = None          # (state, t) of tile awaiting s4..s6
            for t in tseq:
                st = {}
                s_load(st, t)
                s1(st)
                if prev is not None:
                    s4(prev[0])
                s2(st)
                if prev is not None:
                    s5(prev[0])
                s3(st)
                if prev is not None:
                    s6(*prev)
                prev = (st, t)
            s4(prev[0])
            s5(prev[0])
            s6(*prev)

    _split_multiwait_instructions(nc)
    return nc


def make_in_maps(x, winputs, per_core, padded):
    """Per-core input dicts: transposed fp16 x shards + packed weights."""
    n_total = x.shape[0]
    w = _prep_weights(winputs)
    in_maps = []
    for c in range(N_CORES):
        lo = c * per_core
        hi = min(lo + per_core, n_total)
        xc = np.zeros((NF, padded), np.float16)
        xc[:, :hi - lo] = x[lo:hi].T.astype(np.float16)
        in_maps.append({'xin': xc, **w})
    return in_maps


def kernel(**inputs):
    global LAST_RESULT
    x = np.ascontiguousarray(np.asarray(inputs['x'], dtype=np.float32))
    n_total = x.shape[0]
    per_core = (n_total + N_CORES - 1) // N_CORES
    ntiles = (per_core + TILE_PTS - 1) // TILE_PTS
    padded = ntiles * TILE_PTS

    key = padded
    if key not in _PROG_CACHE:
        _PROG_CACHE[key] = _build_program(padded)
    nc = _PROG_CACHE[key]

    in_maps = make_in_maps(
        x, {k: v for k, v in inputs.items() if k != 'x'}, per_core, padded)

    trace = bool(int(os.environ.get('NERF_TRACE', '0')))
    res = run_bass_kernel_spmd(nc, in_maps, list(range(N_CORES)), trace=trace)
    LAST_RESULT = res

    pieces = []
    for c in range(N_CORES):
        lo = c * per_core
        hi = min(lo + per_core, n_total)
        o = res.results[c]['out'].reshape(ntiles, 128, TILE_PTS // 128, 6)
        o = o.transpose(0, 2, 1, 3).reshape(padded, 6)
        pieces.append(o[:hi - lo])
    return np.concatenate(pieces, axis=0)
